# revision 12
# baseline (speedup 1.0000x reference)
"""Trainium2 Bass kernel for nn_IngredientScannerLoss.

Per row (12 coords = 6 (x,y) pairs):
    delta = output - target
    dist_j = sqrt(dx_j^2 + dy_j^2)
    n_j    = (s0_j*dx_j > 0) + (s1_j*dy_j > 0)   (sign-gated count, 0/1/2)
    f(x)   = ((x+1)^1.2 - 1)*2
    t_j    = [dist, f(dist), f(f(dist))][n_j]
    loss   = sum_j t_j

Data-parallel over 8 NeuronCores: rows split 8 x 500_000, each shard
zero-padded to 501_760 = 128*560*7 rows; tiles are [128, 560*12] fp32.

v2 design (per-tile budgets, measured on HW @0.96GHz DVE / 1.2GHz ACT):
  - host stages each (tile, partition) block x/y-SPLIT pair-major
    ([r][6][2] -> [2][6][r]) so every DVE op below reads flat 1D
    unit-stride streams (strided/2D APs cost 1.7-2.4 c/e vs ~1.0).
  - subtract reads fp32 (exact) but writes a bf16 delta tile: sign is
    preserved exactly by RN (flush only below 2^-133 ~ never), so the
    strict gates stay exact while pairdist reads halve.
  - gates: 4 fused 3-slice DVE ops (cmp,cmp,add: 2x-capable shape) on
    the bf16 delta pair slices + 2 tensor_scalar compares for the
    sign-0 pairs; n in fp16 {0,1,2}.  m2 = n&0x4000 (int16 bitcast:
    nonzero iff n==2) -- one cheap TS op, no ACT Relu pass.
  - ACT chain is the 32-unit floor (6 instructions, one ln+exp table):
      lt=ln(s); res=exp(.5lt)=dist; t=ln(res+1); W0=exp(1.2t+ln2)=d1+2
      t2=ln(W0-1); W1=exp(1.2t2+ln2)=d2+2
    The old Copy(bias=-2) un-offset pass (10 units, 5us) is GONE:
    res+=2 (one TS) makes all three candidates uniformly +2, selects
    copy W0/W1 verbatim, and the +12 row offset dies in the sum tree's
    first level via scalar_tensor_tensor((res_lo - 4) + res_hi).
  - emission is software-pipelined (tile i+1 front before tile i back);
    steady state ~ DVE 22us, ACT 17us, DMA 20us per 560-row tile.
"""

import numpy as np

import concourse.bacc as bacc
import concourse.bass as bass
import concourse.mybir as mybir
import concourse.tile as tile
from concourse import dve_ops
from concourse.bass_utils import run_bass_kernel_spmd
from concourse.dve_ops import DveOp
from concourse.dve_spec import Spec, Src0, Src1, Zero, _has_src1, lower, sq
from concourse.dve_uop import (
    AluInp,
    AluOp as UAluOp,
    DelayInp,
    DveOpSpec,
    InpSel,
    OutPath,
    OutSel,
    Trigger,
    UopConfig,
    UopDpConfig,
)

P = 128
COLS = 12
NPAIR = 6
B = 4_000_000
N_CORES = 8
ROWS_VALID = B // N_CORES          # 500_000
RT = 560                           # max rows per partition per tile
NT = 7                             # nominal tiles per core
# edge-ramped tile schedule (sums to RT*NT): smaller first/last tiles
# shorten the software pipeline's fill and drain
RTS = [280, 560, 560, 560, 560, 560, 560, 280]
ROWS_PC = P * RT * NT              # 501_760 padded rows per core
LN2 = 0.6931471805599453

# per-coordinate condition signs (see reference _SIGNS)
SIGNS = [1.0, 1.0, 1.0, -1.0, -1.0, -1.0, -1.0, 1.0, 0.0, 1.0, 0.0, -1.0]

F32 = mybir.dt.float32
F16 = mybir.dt.float16
BF16 = mybir.dt.bfloat16
I16 = mybir.dt.int16
AF = mybir.ActivationFunctionType
ALU = mybir.AluOpType

# ---------------------------------------------------------------- custom ops

USE_2X = True  # hand-authored 2x_1p uOp variants for the 3-slice bodies
_2X_OPS = {"ANT_PAIRDIST", "ANT_GATE_PP", "ANT_GATE_PM",
           "ANT_GATE_MM", "ANT_GATE_MP"}


def _patch_custom_dve_perf():
    """bass's _custom_dve constructs InstCustomDveAnt without perf_max
    (byte-36[7:6]: highest engine-reachable perf table slot) and the
    attribute does not write through after add_instruction. Wrap the
    constructor so our 2x-capable ops are born with perf_max=1."""
    import concourse.bass_isa as bass_isa

    real = bass_isa.InstCustomDveAnt
    if getattr(real, "_ant_2x_wrapped", False):
        return

    def make(*args, **kw):
        if USE_2X and kw.get("op_name") in _2X_OPS:
            kw.setdefault("perf_max", 1)
        return real(*args, **kw)

    make._ant_2x_wrapped = True
    bass_isa.InstCustomDveAnt = make


_patch_custom_dve_perf()


def _register_op(name: str, spec: Spec, subdim: bool = False,
                 uop2x: "UopConfig | None" = None) -> DveOp:
    for op in dve_ops.OPS:
        if op.name == name:
            return op
    if name not in dve_ops._SUB_OPCODE_FOR_NAME:
        row = max(dve_ops._SUB_OPCODE_FOR_NAME.values()) + 1
        assert row < 0x20, "custom DVE opcode rows exhausted"
        dve_ops._SUB_OPCODE_FOR_NAME[name] = row
    shas = {}
    for ver in ("v3", "v4"):
        try:
            shas[ver] = DveOpSpec(
                name=name,
                opcode=dve_ops.get_dve_sub_opcode(name),
                uops=lower(spec, ver=ver),
                rd1_en=_has_src1(spec),
            ).sha(ver)
        except Exception:
            pass
    op = DveOp(name, spec, subdim, shas)
    dve_ops.OPS.append(op)
    dve_ops.CUSTOM_DVE_SPECS[name] = spec
    if uop2x is not None and USE_2X:
        # Pre-seed the compile cache with a spec carrying the 2x_1p
        # program (table slots at table_ptr+1; RTL engages it when all
        # operands are 2B unit-stride). DveOp.compile() consults this
        # cache first, so both the per-NEFF table writer and the emit
        # path see the variant.
        full = DveOpSpec(
            name=name,
            opcode=dve_ops.get_dve_sub_opcode(name),
            uops=lower(spec, ver="v3"),
            uops_2x=[uop2x],
            rd1_en=_has_src1(spec),
            perf_max=1,
        )
        dve_ops._COMPILE_CACHE[(name, "v3")] = full
    return op


def _std_uop() -> UopConfig:
    u = UopConfig()
    u.require_inp0 = 1
    u.require_inp1 = 1
    u.trigger = (Trigger.SRC_TENSOR_DONE, Trigger.NONE, Trigger.NONE)
    u.next_uop = (0, 0, 0)
    u.enable_input(InpSel.SRC_0, 1)     # -> delay_0 (elem A x)
    u.enable_input(InpSel.SRC_1, 2)     # -> delay_1 (elem A y)
    u.enable_input(InpSel.SRC_0_HI, 3)  # -> delay_2 (elem B x)
    u.enable_input(InpSel.SRC_1_HI, 4)  # -> delay_3 (elem B y)
    u.enable_output(OutSel.DELAY_0, OutPath.WR0_LO)   # elem A result
    u.enable_output(OutSel.ALU_OUT, OutPath.WR0_HI)   # elem B result
    return u


def _uop2x_pairdist() -> UopConfig:
    """2x_1p program for s = x^2 + y^2: slices 0-2 compute the packed-lo
    element, 3-5 the packed-hi element; lo result rides delay_0."""
    u = _std_uop()
    dp = u.datapath_config
    dp[0] = (UopDpConfig()
             .enable_alu(UAluOp.MULTIPLY, AluInp.PREV_DELAY_0,
                         AluInp.PREV_DELAY_0)
             .pass_through_delay(1, 2, 3))
    dp[1] = (UopDpConfig()
             .enable_alu(UAluOp.MULTIPLY, AluInp.PREV_DELAY_1,
                         AluInp.PREV_DELAY_1)
             .enable_delay_from_src(DelayInp.PREV_ALU_OUT, 0)
             .pass_through_delay(2, 3))
    dp[2] = (UopDpConfig()
             .enable_alu(UAluOp.ADD, AluInp.PREV_ALU_OUT,
                         AluInp.PREV_DELAY_0)
             .pass_through_delay(2, 3))
    dp[3] = (UopDpConfig()
             .enable_alu(UAluOp.MULTIPLY, AluInp.PREV_DELAY_2,
                         AluInp.PREV_DELAY_2)
             .enable_delay_from_src(DelayInp.PREV_ALU_OUT, 0)
             .pass_through_delay(3))
    dp[4] = (UopDpConfig()
             .enable_alu(UAluOp.MULTIPLY, AluInp.PREV_DELAY_3,
                         AluInp.PREV_DELAY_3)
             .enable_delay_from_src(DelayInp.PREV_ALU_OUT, 1)
             .pass_through_delay(0))
    dp[5] = (UopDpConfig()
             .enable_alu(UAluOp.ADD, AluInp.PREV_ALU_OUT,
                         AluInp.PREV_DELAY_1)
             .pass_through_delay(0))
    dp[6] = (UopDpConfig()
             .enable_alu(UAluOp.BYPASS, AluInp.PREV_ALU_OUT)
             .pass_through_delay(0))
    dp[7] = (UopDpConfig()
             .enable_alu(UAluOp.BYPASS, AluInp.PREV_ALU_OUT)
             .pass_through_delay(0))
    return u


def _uop2x_gate(gt0: bool, gt1: bool) -> UopConfig:
    """2x_1p program for n = cmp0(x) + cmp1(y), cmp = (>0) or (<0)."""
    opx = UAluOp.IS_GT if gt0 else UAluOp.IS_LT
    opy = UAluOp.IS_GT if gt1 else UAluOp.IS_LT
    u = _std_uop()
    u.enable_input(InpSel.ZERO, 5)      # -> delay_4 (constant 0)
    dp = u.datapath_config
    dp[0] = (UopDpConfig()
             .enable_alu(opx, AluInp.PREV_DELAY_0, AluInp.PREV_DELAY_4)
             .pass_through_delay(1, 2, 3, 4))
    dp[1] = (UopDpConfig()
             .enable_alu(opy, AluInp.PREV_DELAY_1, AluInp.PREV_DELAY_4)
             .enable_delay_from_src(DelayInp.PREV_ALU_OUT, 0)
             .pass_through_delay(2, 3, 4))
    dp[2] = (UopDpConfig()
             .enable_alu(UAluOp.ADD, AluInp.PREV_ALU_OUT,
                         AluInp.PREV_DELAY_0)
             .pass_through_delay(2, 3, 4))
    dp[3] = (UopDpConfig()
             .enable_alu(opx, AluInp.PREV_DELAY_2, AluInp.PREV_DELAY_4)
             .enable_delay_from_src(DelayInp.PREV_ALU_OUT, 0)
             .pass_through_delay(3, 4))
    dp[4] = (UopDpConfig()
             .enable_alu(opy, AluInp.PREV_DELAY_3, AluInp.PREV_DELAY_4)
             .enable_delay_from_src(DelayInp.PREV_ALU_OUT, 1)
             .pass_through_delay(0))
    dp[5] = (UopDpConfig()
             .enable_alu(UAluOp.ADD, AluInp.PREV_ALU_OUT,
                         AluInp.PREV_DELAY_1)
             .pass_through_delay(0))
    dp[6] = (UopDpConfig()
             .enable_alu(UAluOp.BYPASS, AluInp.PREV_ALU_OUT)
             .pass_through_delay(0))
    dp[7] = (UopDpConfig()
             .enable_alu(UAluOp.BYPASS, AluInp.PREV_ALU_OUT)
             .pass_through_delay(0))
    return u


# s = in0^2 + in1^2  (in0/in1 = x/y delta streams)
PAIRDIST = _register_op(
    "ANT_PAIRDIST",
    Spec(
        body=sq(Src0) + sq(Src1),
        reference=lambda in0, in1, s0, s1, imm2: (
            in0.astype(np.float32) ** 2 + in1.astype(np.float32) ** 2
        ),
    ),
    uop2x=_uop2x_pairdist(),
)

# n = (sgn0 * in0 > 0) + (sgn1 * in1 > 0), signs baked per-op (3-slice
# bodies: cmp, cmp, add -- keeps the 2x packing door open)


def _gate(name, gt0, gt1):
    c0 = (Src0 > Zero) if gt0 else (Src0 < Zero)
    c1 = (Src1 > Zero) if gt1 else (Src1 < Zero)

    def ref(in0, in1, s0, s1, imm2, _gt0=gt0, _gt1=gt1):
        a = in0.astype(np.float32)
        b = in1.astype(np.float32)
        r0 = (a > 0) if _gt0 else (a < 0)
        r1 = (b > 0) if _gt1 else (b < 0)
        return r0.astype(np.float32) + r1.astype(np.float32)

    return _register_op(name, Spec(body=c0 + c1, reference=ref),
                        uop2x=_uop2x_gate(gt0, gt1))


GATE_PP = _gate("ANT_GATE_PP", True, True)
GATE_PM = _gate("ANT_GATE_PM", True, False)
GATE_MM = _gate("ANT_GATE_MM", False, False)
GATE_MP = _gate("ANT_GATE_MP", False, True)
# pair j -> gate op (pairs 4,5 have sign 0 on x: plain tensor_scalar)
GATE_FOR_PAIR = {0: GATE_PP, 1: GATE_PM, 2: GATE_MM, 3: GATE_MP}


# ---------------------------------------------------------------- act tables
# The stock table-load pass resolves Exp -> exp_and_others and
# Ln -> natural_log, reloading ACT tables on every Ln<->Exp switch
# (~2.7us each). Restrict ln/exp membership to sets that hold BOTH so
# every activation resolves to natural_log_exp_and_others and the load
# hoists to one per kernel. Dict order (act_func_set_id) is preserved.

_GAT_REAL = None


def _gat_lnexp(arch):
    global _GAT_REAL
    from concourse.hw_specs import get_activation_tables

    if _GAT_REAL is None:
        _GAT_REAL = get_activation_tables
    tabs = _GAT_REAL(arch)
    out = {}
    for name, funcs in tabs.items():
        fs = set(funcs)
        if not (AF.Ln in fs and AF.Exp in fs):
            fs.discard(AF.Ln)
            fs.discard(AF.Exp)
        out[name] = fs
    return out


def _patch_act_tables():
    if bacc.get_activation_tables is not _gat_lnexp:
        global _GAT_REAL
        _GAT_REAL = bacc.get_activation_tables
        bacc.get_activation_tables = _gat_lnexp


# ---------------------------------------------------------------- bass build


def build_nc(rt: int = RT, nt: int = NT):
    """Build the single-core SPMD program for [P*rt*nt, 12] inputs."""
    _patch_act_tables()
    rows = P * rt * nt
    rts = RTS if (rt, nt) == (RT, NT) else [rt] * nt
    assert sum(rts) == rt * nt
    w6 = rt * NPAIR          # pair-width (elements)
    w12 = rt * COLS
    nc = bacc.Bacc("TRN2", debug=False, target_bir_lowering=False,
                   num_devices=N_CORES)
    # activation biases need registered const APs (only 0.0/1.0 ship)
    for cv in (-1.0, LN2):
        if (F32, cv) not in nc.const_aps.aps:
            ct = nc.alloc_sbuf_tensor(f"const-f32-{cv}", [P, 1], F32)
            nc.gpsimd.memset(ct.ap(), cv)
            nc.const_aps.aps[(F32, cv)] = ct.ap()
    nc.all_engine_barrier()
    a = nc.dram_tensor("output", [rows, COLS], F32, kind="ExternalInput").ap()
    b = nc.dram_tensor("target", [rows, COLS], F32, kind="ExternalInput").ap()
    o = nc.dram_tensor("loss", [rows], F32, kind="ExternalOutput").ap()

    def emit_front(pool, rti, an, bn):
        """Loads + delta + squares + gates + full ACT chain for one tile.
        Returns the state the back-end phase needs."""
        u6 = rti * NPAIR
        u12 = rti * COLS
        u4 = rti * 4

        ta = pool.tile([P, w12], F32, tag="ta")
        nc.sync.dma_start(out=ta[:, 0:u12], in_=an)
        tb = pool.tile([P, w12], F32, tag="tb")
        nc.sync.dma_start(out=tb[:, 0:u12], in_=bn)

        # ---- delta = a - b: fp32 reads (exact signs survive the bf16
        # RN write), bf16 out so downstream reads are 2B
        dl = pool.tile([P, w12], BF16, tag="dl")
        nc.vector.tensor_tensor(dl[:, 0:u12], ta[:, 0:u12],
                                tb[:, 0:u12], ALU.subtract)
        dx = dl[:, 0:u6]             # x block, pair-major
        dy = dl[:, u6:2 * u6]        # y block, pair-major

        # ---- s = dx^2 + dy^2, pair-major fp16, flat 1D streams
        slt = pool.tile([P, w6], F16, tag="slt")
        pd = nc.vector._custom_dve(PAIRDIST, out=slt[:, 0:u6],
                                   in0=dx, in1=dy)
        if USE_2X:
            pd.perf_max = 1

        # ---- gates -> n in {0,1,2} fp16, pair-major (flat 1D streams)
        n16 = pool.tile([P, w6], F16, tag="n16")
        for j in range(NPAIR):
            xs = slice(j * rti, (j + 1) * rti)
            if SIGNS[2 * j] != 0.0:
                gi = nc.vector._custom_dve(
                    GATE_FOR_PAIR[j], out=n16[:, xs],
                    in0=dx[:, xs], in1=dy[:, xs],
                )
                if USE_2X:
                    gi.perf_max = 1
            else:
                op = ALU.is_gt if SIGNS[2 * j + 1] > 0 else ALU.is_lt
                nc.vector.tensor_scalar(n16[:, xs], dy[:, xs],
                                        0.0, None, op)
        # m2 nonzero iff n == 2 (fp16 {0,1,2} -> 0x0000/0x3C00/0x4000)
        m2t = pool.tile([P, rt * 4], I16, tag="m2t")
        nc.vector.tensor_scalar(m2t[:, 0:u4], n16[:, 0:u4].bitcast(I16),
                                0x4000, None, ALU.bitwise_and)

        # ---- ACT chain, one table set (ln+exp), all pair-major
        # contiguous; t2/W1 on the pairs-0..3 prefix (pairs 4,5 have
        # max n == 1). Candidates are kept at +2 offset (W0 = d1+2,
        # W1 = d2+2, res+2 after its last ACT read) -- no un-offset
        # Copy pass; the row sum subtracts the uniform 12 in the tree.
        #   lt  = ln(s)            (in-place on slt)
        #   res = exp(0.5*lt)      = dist
        #   t   = ln(res + 1)      (in-place on slt)
        #   W0  = exp(1.2*t + ln2) = d1 + 2
        #   t2  = ln(W0 - 1)       (in-place on slt prefix)
        #   W1  = exp(1.2*t2+ln2)  = d2 + 2
        nc.scalar.activation(slt[:, 0:u6], slt[:, 0:u6], AF.Ln)
        res = pool.tile([P, w6], F16, tag="res")
        nc.scalar.activation(res[:, 0:u6], slt[:, 0:u6], AF.Exp, scale=0.5)
        nc.scalar.activation(slt[:, 0:u6], res[:, 0:u6], AF.Ln, bias=1.0)
        w0 = pool.tile([P, w6], F16, tag="w0")
        nc.scalar.activation(w0[:, 0:u6], slt[:, 0:u6], AF.Exp,
                             scale=1.2, bias=LN2)
        nc.scalar.activation(slt[:, 0:u4], w0[:, 0:u4], AF.Ln, bias=-1.0)
        w1 = pool.tile([P, rt * 4], F16, tag="w1")
        nc.scalar.activation(w1[:, 0:u4], slt[:, 0:u4], AF.Exp,
                             scale=1.2, bias=LN2)
        # res -> dist + 2 (after the Ln(res+1) read)
        nc.vector.tensor_scalar(res[:, 0:u6], res[:, 0:u6], 2.0, None,
                                ALU.add)
        return dict(rti=rti, res=res, n16=n16, w0=w0, w1=w1, m2t=m2t)

    def emit_back(pool, st, on):
        """Selects + row-sum tree + output DMA for one tile."""
        rti = st["rti"]
        u6 = rti * NPAIR
        u4 = rti * 4
        u3 = rti * 3
        res, n16, w0, w1, m2t = (st["res"], st["n16"], st["w0"],
                                 st["w1"], st["m2t"])

        # ---- select: res (dist+2) overwritten by W0 (d1+2) where n>=1,
        # W1 (d2+2) where n==2. All APs pair-major contiguous fp16.
        nc.vector.copy_predicated(res[:, 0:u6],
                                  n16[:, 0:u6].bitcast(I16), w0[:, 0:u6])
        nc.vector.copy_predicated(res[:, 0:u4], m2t[:, 0:u4], w1[:, 0:u4])

        # ---- row sums: contiguous fp16 tree over pairs (plain TT adds
        # run 2x on fp16; STT does not, so the uniform +12-per-row
        # offset is folded into the narrow fp32 last level instead).
        nc.vector.tensor_tensor(res[:, 0:u3], res[:, 0:u3],
                                res[:, u3:u6], ALU.add)
        nc.vector.tensor_tensor(res[:, 0:rti], res[:, 0:rti],
                                res[:, rti:2 * rti], ALU.add)
        ot = pool.tile([P, RT], F32, tag="ot")
        nc.vector.scalar_tensor_tensor(ot[:, 0:rti], res[:, 0:rti], -12.0,
                                       res[:, 2 * rti:u3], ALU.add, ALU.add)
        nc.sync.dma_start(out=on, in_=ot[:, 0:rti])

    # Software-pipelined emission: tile i+1's front-end is emitted BEFORE
    # tile i's back-end, so the in-order DVE queue never blocks ready
    # front-end work behind a select that is waiting on the ACT chain.
    with tile.TileContext(nc) as tc:
        with tc.tile_pool(name="sb", bufs=2) as pool:
            off = 0
            pending = None
            for ti, rti in enumerate(rts):
                an = a[off * P:(off + rti) * P].rearrange(
                    "(p r) m -> p (r m)", p=P)
                bn = b[off * P:(off + rti) * P].rearrange(
                    "(p r) m -> p (r m)", p=P)
                on = o[off * P:(off + rti) * P].rearrange(
                    "(p r) -> p r", p=P)
                off += rti
                st = emit_front(pool, rti, an, bn)
                if pending is not None:
                    emit_back(pool, pending[0], pending[1])
                pending = (st, on)
            emit_back(pool, pending[0], pending[1])
    nc.compile()
    return nc


_NC_CACHE: dict = {}


def _get_nc(rt: int = RT, nt: int = NT):
    key = (rt, nt)
    if key not in _NC_CACHE:
        _NC_CACHE[key] = build_nc(rt, nt)
    return _NC_CACHE[key]


# ---------------------------------------------------------------- entrypoint


def _stage(x_core):
    """Permute one core's padded [ROWS_PC, 12] block so each (tile,
    partition) region is x/y-split pair-major: [rt, 6, 2] -> [2, 6, rt].
    Pure layout staging (bytes reordered, values untouched); the device
    DMA still copies contiguous per-partition ranges."""
    out = np.empty_like(x_core)
    off = 0
    for rt_i in RTS:
        n = P * rt_i
        blk = x_core[off:off + n].reshape(P, rt_i, NPAIR, 2)
        out[off:off + n] = blk.transpose(0, 3, 2, 1).reshape(n, COLS)
        off += n
    return out


def make_in_maps(a, b):
    a_sh = np.zeros((N_CORES, ROWS_PC, COLS), dtype=np.float32)
    b_sh = np.zeros((N_CORES, ROWS_PC, COLS), dtype=np.float32)
    a_sh[:, :ROWS_VALID, :] = a.reshape(N_CORES, ROWS_VALID, COLS)
    b_sh[:, :ROWS_VALID, :] = b.reshape(N_CORES, ROWS_VALID, COLS)
    return [
        {"output": _stage(a_sh[c]), "target": _stage(b_sh[c])}
        for c in range(N_CORES)
    ]


def kernel(output, target):
    a = np.asarray(output, dtype=np.float32)
    b = np.asarray(target, dtype=np.float32)
    assert a.shape == (B, COLS) and b.shape == (B, COLS)

    nc = _get_nc()
    in_maps = make_in_maps(a, b)
    r = run_bass_kernel_spmd(nc, in_maps, list(range(N_CORES)))
    out = np.empty((N_CORES, ROWS_VALID), dtype=np.float32)
    for c in range(N_CORES):
        out[c] = r.results[c]["loss"][:ROWS_VALID]
    return out.reshape(B)


# revision 21
# speedup vs baseline: 1.1339x; 1.1339x over previous
"""Trainium2 Bass kernel for nn_IngredientScannerLoss.

Per row (12 coords = 6 (x,y) pairs):
    delta = output - target
    dist_j = sqrt(dx_j^2 + dy_j^2)
    n_j    = (s0_j*dx_j > 0) + (s1_j*dy_j > 0)   (sign-gated count, 0/1/2)
    f(x)   = ((x+1)^1.2 - 1)*2
    t_j    = [dist, f(dist), f(f(dist))][n_j]
    loss   = sum_j t_j

Data-parallel over 8 NeuronCores: rows split 8 x 500_000, each shard
zero-padded to 501_760 = 128*560*7 rows; tiles are [128, 560*12] fp32.

v2 design (per-tile budgets, measured on HW @0.96GHz DVE / 1.2GHz ACT):
  - host stages each (tile, partition) block x/y-SPLIT pair-major
    ([r][6][2] -> [2][6][r]) so every DVE op below reads flat 1D
    unit-stride streams (strided/2D APs cost 1.7-2.4 c/e vs ~1.0).
  - subtract reads fp32 (exact) but writes a bf16 delta tile: sign is
    preserved exactly by RN (flush only below 2^-133 ~ never), so the
    strict gates stay exact while pairdist reads halve.
  - gates: 4 fused 3-slice DVE ops (cmp,cmp,add: 2x-capable shape) on
    the bf16 delta pair slices + 2 tensor_scalar compares for the
    sign-0 pairs; n in fp16 {0,1,2}.  m2 = n&0x4000 (int16 bitcast:
    nonzero iff n==2) -- one cheap TS op, no ACT Relu pass.
  - ACT chain is the 32-unit floor (6 instructions, one ln+exp table):
      lt=ln(s); res=exp(.5lt)=dist; t=ln(res+1); W0=exp(1.2t+ln2)=d1+2
      t2=ln(W0-1); W1=exp(1.2t2+ln2)=d2+2
    The old Copy(bias=-2) un-offset pass (10 units, 5us) is GONE:
    res+=2 (one TS) makes all three candidates uniformly +2, selects
    copy W0/W1 verbatim, and the +12 row offset dies in the sum tree's
    first level via scalar_tensor_tensor((res_lo - 4) + res_hi).
  - emission is software-pipelined (tile i+1 front before tile i back);
    steady state ~ DVE 22us, ACT 17us, DMA 20us per 560-row tile.
"""

import numpy as np

import concourse.bacc as bacc
import concourse.bass as bass
import concourse.mybir as mybir
import concourse.tile as tile
from concourse import dve_ops
from concourse.bass_utils import run_bass_kernel_spmd
from concourse.dve_ops import DveOp
from concourse.dve_spec import Spec, Src0, Src1, Zero, _has_src1, lower, sq
from concourse.dve_uop import (
    AluInp,
    AluOp as UAluOp,
    DelayInp,
    DveOpSpec,
    InpSel,
    OutPath,
    OutSel,
    Trigger,
    UopConfig,
    UopDpConfig,
)

P = 128
COLS = 12
NPAIR = 6
B = 4_000_000
N_CORES = 8
ROWS_VALID = B // N_CORES          # 500_000
RT = 560                           # max rows per partition per tile
NT = 7                             # nominal tiles per core
# edge-ramped tile schedule (sums to RT*NT): smaller first/last tiles
# shorten the software pipeline's fill and drain
RTS = [140, 420, 560, 560, 560, 560, 560, 420, 140]
ROWS_PC = P * RT * NT              # 501_760 padded rows per core
LN2 = 0.6931471805599453

# per-coordinate condition signs (see reference _SIGNS)
SIGNS = [1.0, 1.0, 1.0, -1.0, -1.0, -1.0, -1.0, 1.0, 0.0, 1.0, 0.0, -1.0]

F32 = mybir.dt.float32
F16 = mybir.dt.float16
BF16 = mybir.dt.bfloat16
I16 = mybir.dt.int16
AF = mybir.ActivationFunctionType
ALU = mybir.AluOpType

# ---------------------------------------------------------------- custom ops

USE_2X = True  # hand-authored 2x_1p uOp variants for the 3-slice bodies
_2X_OPS = {"ANT_PAIRDIST", "ANT_GATE_PP", "ANT_GATE_PM",
           "ANT_GATE_MM", "ANT_GATE_MP"}


def _patch_custom_dve_perf():
    """bass's _custom_dve constructs InstCustomDveAnt without perf_max
    (byte-36[7:6]: highest engine-reachable perf table slot) and the
    attribute does not write through after add_instruction. Wrap the
    constructor so our 2x-capable ops are born with perf_max=1."""
    import concourse.bass_isa as bass_isa

    real = bass_isa.InstCustomDveAnt
    if getattr(real, "_ant_2x_wrapped", False):
        return

    def make(*args, **kw):
        if USE_2X and kw.get("op_name") in _2X_OPS:
            kw.setdefault("perf_max", 1)
        return real(*args, **kw)

    make._ant_2x_wrapped = True
    bass_isa.InstCustomDveAnt = make


_patch_custom_dve_perf()


def _register_op(name: str, spec: Spec, subdim: bool = False,
                 uop2x: "UopConfig | None" = None) -> DveOp:
    for op in dve_ops.OPS:
        if op.name == name:
            return op
    if name not in dve_ops._SUB_OPCODE_FOR_NAME:
        row = max(dve_ops._SUB_OPCODE_FOR_NAME.values()) + 1
        assert row < 0x20, "custom DVE opcode rows exhausted"
        dve_ops._SUB_OPCODE_FOR_NAME[name] = row
    shas = {}
    for ver in ("v3", "v4"):
        try:
            shas[ver] = DveOpSpec(
                name=name,
                opcode=dve_ops.get_dve_sub_opcode(name),
                uops=lower(spec, ver=ver),
                rd1_en=_has_src1(spec),
            ).sha(ver)
        except Exception:
            pass
    op = DveOp(name, spec, subdim, shas)
    dve_ops.OPS.append(op)
    dve_ops.CUSTOM_DVE_SPECS[name] = spec
    if uop2x is not None and USE_2X:
        # Pre-seed the compile cache with a spec carrying the 2x_1p
        # program (table slots at table_ptr+1; RTL engages it when all
        # operands are 2B unit-stride). DveOp.compile() consults this
        # cache first, so both the per-NEFF table writer and the emit
        # path see the variant.
        full = DveOpSpec(
            name=name,
            opcode=dve_ops.get_dve_sub_opcode(name),
            uops=lower(spec, ver="v3"),
            uops_2x=[uop2x],
            rd1_en=_has_src1(spec),
            perf_max=1,
        )
        dve_ops._COMPILE_CACHE[(name, "v3")] = full
    return op


def _std_uop() -> UopConfig:
    u = UopConfig()
    u.require_inp0 = 1
    u.require_inp1 = 1
    u.trigger = (Trigger.SRC_TENSOR_DONE, Trigger.NONE, Trigger.NONE)
    u.next_uop = (0, 0, 0)
    u.enable_input(InpSel.SRC_0, 1)     # -> delay_0 (elem A x)
    u.enable_input(InpSel.SRC_1, 2)     # -> delay_1 (elem A y)
    u.enable_input(InpSel.SRC_0_HI, 3)  # -> delay_2 (elem B x)
    u.enable_input(InpSel.SRC_1_HI, 4)  # -> delay_3 (elem B y)
    u.enable_output(OutSel.DELAY_0, OutPath.WR0_LO)   # elem A result
    u.enable_output(OutSel.ALU_OUT, OutPath.WR0_HI)   # elem B result
    return u


def _uop2x_pairdist() -> UopConfig:
    """2x_1p program for s = x^2 + y^2: slices 0-2 compute the packed-lo
    element, 3-5 the packed-hi element; lo result rides delay_0."""
    u = _std_uop()
    dp = u.datapath_config
    dp[0] = (UopDpConfig()
             .enable_alu(UAluOp.MULTIPLY, AluInp.PREV_DELAY_0,
                         AluInp.PREV_DELAY_0)
             .pass_through_delay(1, 2, 3))
    dp[1] = (UopDpConfig()
             .enable_alu(UAluOp.MULTIPLY, AluInp.PREV_DELAY_1,
                         AluInp.PREV_DELAY_1)
             .enable_delay_from_src(DelayInp.PREV_ALU_OUT, 0)
             .pass_through_delay(2, 3))
    dp[2] = (UopDpConfig()
             .enable_alu(UAluOp.ADD, AluInp.PREV_ALU_OUT,
                         AluInp.PREV_DELAY_0)
             .pass_through_delay(2, 3))
    dp[3] = (UopDpConfig()
             .enable_alu(UAluOp.MULTIPLY, AluInp.PREV_DELAY_2,
                         AluInp.PREV_DELAY_2)
             .enable_delay_from_src(DelayInp.PREV_ALU_OUT, 0)
             .pass_through_delay(3))
    dp[4] = (UopDpConfig()
             .enable_alu(UAluOp.MULTIPLY, AluInp.PREV_DELAY_3,
                         AluInp.PREV_DELAY_3)
             .enable_delay_from_src(DelayInp.PREV_ALU_OUT, 1)
             .pass_through_delay(0))
    dp[5] = (UopDpConfig()
             .enable_alu(UAluOp.ADD, AluInp.PREV_ALU_OUT,
                         AluInp.PREV_DELAY_1)
             .pass_through_delay(0))
    dp[6] = (UopDpConfig()
             .enable_alu(UAluOp.BYPASS, AluInp.PREV_ALU_OUT)
             .pass_through_delay(0))
    dp[7] = (UopDpConfig()
             .enable_alu(UAluOp.BYPASS, AluInp.PREV_ALU_OUT)
             .pass_through_delay(0))
    return u


def _uop1x_cpred_ne(const_lane: InpSel = InpSel.CONST_0) -> UopConfig:
    """Clone of the stock COPY_PREDICATED uop (data rides delay_1 to the
    write mux, stage-0 ALU result is the write predicate, write fires
    iff the predicate is 0) with the predicate generalized to
    IS_NE(in0, s0): dst = in1 where in0 == s0."""
    u = UopConfig()
    u.require_inp0 = 1
    u.require_inp1 = 1
    u.trigger = (Trigger.SRC_TENSOR_DONE, Trigger.NONE, Trigger.NONE)
    u.next_uop = (0, 0, 0)
    u.enable_input(InpSel.SRC_0, 0)       # mask value -> ALU lane
    u.enable_input(const_lane, 1)         # threshold  -> delay_0
    u.enable_input(InpSel.SRC_1, 2)       # data       -> delay_1
    u.enable_output(OutSel.DELAY_1, OutPath.WR0_LO)
    u.write_predicate_enable = 1
    u.write_predicate_select = 0
    dp = u.datapath_config
    dp[0] = (UopDpConfig()
             .enable_alu(UAluOp.IS_NE, AluInp.PREV_ALU_OUT,
                         AluInp.PREV_DELAY_0)
             .pass_through_delay(1))
    for s in range(1, 8):
        dp[s] = UopDpConfig().pass_through_delay(1)
    return u


def _uop2x_gate(gt0: bool, gt1: bool) -> UopConfig:
    """2x_1p program for n = cmp0(x) + cmp1(y), cmp = (>0) or (<0)."""
    opx = UAluOp.IS_GT if gt0 else UAluOp.IS_LT
    opy = UAluOp.IS_GT if gt1 else UAluOp.IS_LT
    u = _std_uop()
    u.enable_input(InpSel.ZERO, 5)      # -> delay_4 (constant 0)
    dp = u.datapath_config
    dp[0] = (UopDpConfig()
             .enable_alu(opx, AluInp.PREV_DELAY_0, AluInp.PREV_DELAY_4)
             .pass_through_delay(1, 2, 3, 4))
    dp[1] = (UopDpConfig()
             .enable_alu(opy, AluInp.PREV_DELAY_1, AluInp.PREV_DELAY_4)
             .enable_delay_from_src(DelayInp.PREV_ALU_OUT, 0)
             .pass_through_delay(2, 3, 4))
    dp[2] = (UopDpConfig()
             .enable_alu(UAluOp.ADD, AluInp.PREV_ALU_OUT,
                         AluInp.PREV_DELAY_0)
             .pass_through_delay(2, 3, 4))
    dp[3] = (UopDpConfig()
             .enable_alu(opx, AluInp.PREV_DELAY_2, AluInp.PREV_DELAY_4)
             .enable_delay_from_src(DelayInp.PREV_ALU_OUT, 0)
             .pass_through_delay(3, 4))
    dp[4] = (UopDpConfig()
             .enable_alu(opy, AluInp.PREV_DELAY_3, AluInp.PREV_DELAY_4)
             .enable_delay_from_src(DelayInp.PREV_ALU_OUT, 1)
             .pass_through_delay(0))
    dp[5] = (UopDpConfig()
             .enable_alu(UAluOp.ADD, AluInp.PREV_ALU_OUT,
                         AluInp.PREV_DELAY_1)
             .pass_through_delay(0))
    dp[6] = (UopDpConfig()
             .enable_alu(UAluOp.BYPASS, AluInp.PREV_ALU_OUT)
             .pass_through_delay(0))
    dp[7] = (UopDpConfig()
             .enable_alu(UAluOp.BYPASS, AluInp.PREV_ALU_OUT)
             .pass_through_delay(0))
    return u


# s = in0^2 + in1^2  (in0/in1 = x/y delta streams)
PAIRDIST = _register_op(
    "ANT_PAIRDIST",
    Spec(
        body=sq(Src0) + sq(Src1),
        reference=lambda in0, in1, s0, s1, imm2: (
            in0.astype(np.float32) ** 2 + in1.astype(np.float32) ** 2
        ),
    ),
    uop2x=_uop2x_pairdist(),
)

# n = (sgn0 * in0 > 0) + (sgn1 * in1 > 0), signs baked per-op (3-slice
# bodies: cmp, cmp, add -- keeps the 2x packing door open)


def _gate(name, gt0, gt1):
    c0 = (Src0 > Zero) if gt0 else (Src0 < Zero)
    c1 = (Src1 > Zero) if gt1 else (Src1 < Zero)

    def ref(in0, in1, s0, s1, imm2, _gt0=gt0, _gt1=gt1):
        a = in0.astype(np.float32)
        b = in1.astype(np.float32)
        r0 = (a > 0) if _gt0 else (a < 0)
        r1 = (b > 0) if _gt1 else (b < 0)
        return r0.astype(np.float32) + r1.astype(np.float32)

    return _register_op(name, Spec(body=c0 + c1, reference=ref),
                        uop2x=_uop2x_gate(gt0, gt1))


GATE_PP = _gate("ANT_GATE_PP", True, True)
GATE_PM = _gate("ANT_GATE_PM", True, False)
GATE_MM = _gate("ANT_GATE_MM", False, False)
GATE_MP = _gate("ANT_GATE_MP", False, True)
# pair j -> gate op (pairs 4,5 have sign 0 on x: plain tensor_scalar)
GATE_FOR_PAIR = {0: GATE_PP, 1: GATE_PM, 2: GATE_MM, 3: GATE_MP}


def _register_cpred2() -> DveOp:
    """dst = in1 where in0 == s0 (predicated write, dst else untouched).
    The uop program is hand-built (lower() cannot express write
    predication), seeded straight into the compile cache."""
    name = "ANT_CPRED_EQ"
    for op in dve_ops.OPS:
        if op.name == name:
            return op
    if name not in dve_ops._SUB_OPCODE_FOR_NAME:
        row = max(dve_ops._SUB_OPCODE_FOR_NAME.values()) + 1
        assert row < 0x20
        dve_ops._SUB_OPCODE_FOR_NAME[name] = row

    def ref(in0, in1, s0, s1, imm2):
        return np.where(in0 == s0, in1, np.nan)  # sim-only; never simmed

    spec = Spec(body=Src0 + Src1, reference=ref)
    op = DveOp(name, spec, False, {})
    dve_ops.OPS.append(op)
    dve_ops.CUSTOM_DVE_SPECS[name] = spec
    manual = DveOpSpec(
        name=name,
        opcode=dve_ops.get_dve_sub_opcode(name),
        uops=[_uop1x_cpred_ne()],
        rd1_en=True,
    )
    dve_ops._COMPILE_CACHE[(name, "v3")] = manual
    return op


CPRED_EQ = _register_cpred2()


# ---------------------------------------------------------------- act tables
# The stock table-load pass resolves Exp -> exp_and_others and
# Ln -> natural_log, reloading ACT tables on every Ln<->Exp switch
# (~2.7us each). Restrict ln/exp membership to sets that hold BOTH so
# every activation resolves to natural_log_exp_and_others and the load
# hoists to one per kernel. Dict order (act_func_set_id) is preserved.

_GAT_REAL = None


def _gat_lnexp(arch):
    global _GAT_REAL
    from concourse.hw_specs import get_activation_tables

    if _GAT_REAL is None:
        _GAT_REAL = get_activation_tables
    tabs = _GAT_REAL(arch)
    out = {}
    for name, funcs in tabs.items():
        fs = set(funcs)
        if not (AF.Ln in fs and AF.Exp in fs):
            fs.discard(AF.Ln)
            fs.discard(AF.Exp)
        out[name] = fs
    return out


def _patch_act_tables():
    if bacc.get_activation_tables is not _gat_lnexp:
        global _GAT_REAL
        _GAT_REAL = bacc.get_activation_tables
        bacc.get_activation_tables = _gat_lnexp


# ---------------------------------------------------------------- bass build


def build_nc(rt: int = RT, nt: int = NT):
    """Build the single-core SPMD program for [P*rt*nt, 12] inputs."""
    _patch_act_tables()
    rows = P * rt * nt
    rts = RTS if (rt, nt) == (RT, NT) else [rt] * nt
    assert sum(rts) == rt * nt
    w6 = rt * NPAIR          # pair-width (elements)
    w12 = rt * COLS
    nc = bacc.Bacc("TRN2", debug=False, target_bir_lowering=False,
                   num_devices=N_CORES)
    # activation biases need registered const APs (only 0.0/1.0 ship)
    for cv in (-1.0, LN2):
        if (F32, cv) not in nc.const_aps.aps:
            ct = nc.alloc_sbuf_tensor(f"const-f32-{cv}", [P, 1], F32)
            nc.gpsimd.memset(ct.ap(), cv)
            nc.const_aps.aps[(F32, cv)] = ct.ap()
    nc.all_engine_barrier()
    a = nc.dram_tensor("output", [rows, COLS], F32, kind="ExternalInput").ap()
    b = nc.dram_tensor("target", [rows, COLS], F32, kind="ExternalInput").ap()
    o = nc.dram_tensor("loss", [rows], F32, kind="ExternalOutput").ap()

    def emit_front(pool, rti, an, bn):
        """Loads + delta + squares + gates + full ACT chain for one tile.
        Returns the state the back-end phase needs."""
        u6 = rti * NPAIR
        u12 = rti * COLS
        u4 = rti * 4

        ta = pool.tile([P, w12], F32, tag="ta")
        nc.sync.dma_start(out=ta[:, 0:u12], in_=an)
        tb = pool.tile([P, w12], F32, tag="tb")
        nc.sync.dma_start(out=tb[:, 0:u12], in_=bn)

        # ---- delta = a - b: fp32 reads (exact signs survive the bf16
        # RN write), bf16 out so downstream reads are 2B
        dl = pool.tile([P, w12], BF16, tag="dl")
        nc.vector.tensor_tensor(dl[:, 0:u12], ta[:, 0:u12],
                                tb[:, 0:u12], ALU.subtract)
        dx = dl[:, 0:u6]             # x block, pair-major
        dy = dl[:, u6:2 * u6]        # y block, pair-major

        # ---- s = dx^2 + dy^2, pair-major fp16, flat 1D streams
        slt = pool.tile([P, w6], F16, tag="slt")
        pd = nc.vector._custom_dve(PAIRDIST, out=slt[:, 0:u6],
                                   in0=dx, in1=dy)
        if USE_2X:
            pd.perf_max = 1

        # ---- gates -> n in {0,1,2} fp16, pair-major (flat 1D streams)
        n16 = pool.tile([P, w6], F16, tag="n16")
        for j in range(NPAIR):
            xs = slice(j * rti, (j + 1) * rti)
            if SIGNS[2 * j] != 0.0:
                gi = nc.vector._custom_dve(
                    GATE_FOR_PAIR[j], out=n16[:, xs],
                    in0=dx[:, xs], in1=dy[:, xs],
                )
                if USE_2X:
                    gi.perf_max = 1
            else:
                op = ALU.is_gt if SIGNS[2 * j + 1] > 0 else ALU.is_lt
                nc.vector.tensor_scalar(n16[:, xs], dy[:, xs],
                                        0.0, None, op)
        # m2 nonzero iff n == 2 (fp16 {0,1,2} -> 0x0000/0x3C00/0x4000)
        m2t = pool.tile([P, rt * 4], I16, tag="m2t")
        nc.vector.tensor_scalar(m2t[:, 0:u4], n16[:, 0:u4].bitcast(I16),
                                0x4000, None, ALU.bitwise_and)

        # ---- ACT chain, one table set (ln+exp), all pair-major
        # contiguous; t2/W1 on the pairs-0..3 prefix (pairs 4,5 have
        # max n == 1). Candidates are kept at +2 offset (W0 = d1+2,
        # W1 = d2+2, res+2 after its last ACT read) -- no un-offset
        # Copy pass; the row sum subtracts the uniform 12 in the tree.
        #   lt  = ln(s)            (in-place on slt)
        #   res = exp(0.5*lt)      = dist
        #   t   = ln(res + 1)      (in-place on slt)
        #   W0  = exp(1.2*t + ln2) = d1 + 2
        #   t2  = ln(W0 - 1)       (in-place on slt prefix)
        #   W1  = exp(1.2*t2+ln2)  = d2 + 2
        nc.scalar.activation(slt[:, 0:u6], slt[:, 0:u6], AF.Ln)
        res = pool.tile([P, w6], F16, tag="res")
        nc.scalar.activation(res[:, 0:u6], slt[:, 0:u6], AF.Exp, scale=0.5)
        nc.scalar.activation(slt[:, 0:u6], res[:, 0:u6], AF.Ln, bias=1.0)
        w0 = pool.tile([P, w6], F16, tag="w0")
        nc.scalar.activation(w0[:, 0:u6], slt[:, 0:u6], AF.Exp,
                             scale=1.2, bias=LN2)
        nc.scalar.activation(slt[:, 0:u4], w0[:, 0:u4], AF.Ln, bias=-1.0)
        w1 = pool.tile([P, rt * 4], F16, tag="w1")
        nc.scalar.activation(w1[:, 0:u4], slt[:, 0:u4], AF.Exp,
                             scale=1.2, bias=LN2)
        return dict(rti=rti, res=res, n16=n16, w0=w0, w1=w1, m2t=m2t)

    def emit_back(pool, st, on):
        """Selects + row-sum tree + output DMA for one tile."""
        rti = st["rti"]
        u6 = rti * NPAIR
        u4 = rti * 4
        u3 = rti * 3
        res, n16, w0, w1, m2t = (st["res"], st["n16"], st["w0"],
                                 st["w1"], st["m2t"])

        # res -> dist + 2 (emitted in the BACK phase: it depends on the
        # tile's 3rd ACT pass, so emitting it in the front phase would
        # stall the in-order DVE queue ahead of ready select work)
        nc.vector.tensor_scalar(res[:, 0:u6], res[:, 0:u6], 2.0, None,
                                ALU.add)
        # ---- select: res (dist+2) overwritten by W0 (d1+2) where n>=1,
        # W1 (d2+2) where n==2. All APs pair-major contiguous.
        nc.vector.copy_predicated(res[:, 0:u6],
                                  n16[:, 0:u6].bitcast(I16), w0[:, 0:u6])
        nc.vector.copy_predicated(res[:, 0:u4], m2t[:, 0:u4], w1[:, 0:u4])

        # ---- row sums: contiguous fp16 tree over pairs (plain TT adds
        # run 2x on fp16; STT does not, so the uniform +12-per-row
        # offset is folded into the narrow fp32 last level instead).
        nc.vector.tensor_tensor(res[:, 0:u3], res[:, 0:u3],
                                res[:, u3:u6], ALU.add)
        nc.vector.tensor_tensor(res[:, 0:rti], res[:, 0:rti],
                                res[:, rti:2 * rti], ALU.add)
        ot = pool.tile([P, RT], F32, tag="ot")
        nc.vector.scalar_tensor_tensor(ot[:, 0:rti], res[:, 0:rti], -12.0,
                                       res[:, 2 * rti:u3], ALU.add, ALU.add)
        nc.sync.dma_start(out=on, in_=ot[:, 0:rti])

    # Software-pipelined emission: tile i+1's front-end is emitted BEFORE
    # tile i's back-end, so the in-order DVE queue never blocks ready
    # front-end work behind a select that is waiting on the ACT chain.
    with tile.TileContext(nc) as tc:
        with tc.tile_pool(name="sb", bufs=2) as pool:
            off = 0
            pending = None
            for ti, rti in enumerate(rts):
                an = a[off * P:(off + rti) * P].rearrange(
                    "(p r) m -> p (r m)", p=P)
                bn = b[off * P:(off + rti) * P].rearrange(
                    "(p r) m -> p (r m)", p=P)
                on = o[off * P:(off + rti) * P].rearrange(
                    "(p r) -> p r", p=P)
                off += rti
                st = emit_front(pool, rti, an, bn)
                if pending is not None:
                    emit_back(pool, pending[0], pending[1])
                pending = (st, on)
            emit_back(pool, pending[0], pending[1])
    nc.compile()
    return nc


_NC_CACHE: dict = {}


def _get_nc(rt: int = RT, nt: int = NT):
    key = (rt, nt)
    if key not in _NC_CACHE:
        _NC_CACHE[key] = build_nc(rt, nt)
    return _NC_CACHE[key]


# ---------------------------------------------------------------- entrypoint


def _stage(x_core):
    """Permute one core's padded [ROWS_PC, 12] block so each (tile,
    partition) region is x/y-split pair-major: [rt, 6, 2] -> [2, 6, rt].
    Pure layout staging (bytes reordered, values untouched); the device
    DMA still copies contiguous per-partition ranges."""
    out = np.empty_like(x_core)
    off = 0
    for rt_i in RTS:
        n = P * rt_i
        blk = x_core[off:off + n].reshape(P, rt_i, NPAIR, 2)
        out[off:off + n] = blk.transpose(0, 3, 2, 1).reshape(n, COLS)
        off += n
    return out


def make_in_maps(a, b):
    a_sh = np.zeros((N_CORES, ROWS_PC, COLS), dtype=np.float32)
    b_sh = np.zeros((N_CORES, ROWS_PC, COLS), dtype=np.float32)
    a_sh[:, :ROWS_VALID, :] = a.reshape(N_CORES, ROWS_VALID, COLS)
    b_sh[:, :ROWS_VALID, :] = b.reshape(N_CORES, ROWS_VALID, COLS)
    return [
        {"output": _stage(a_sh[c]), "target": _stage(b_sh[c])}
        for c in range(N_CORES)
    ]


def kernel(output, target):
    a = np.asarray(output, dtype=np.float32)
    b = np.asarray(target, dtype=np.float32)
    assert a.shape == (B, COLS) and b.shape == (B, COLS)

    nc = _get_nc()
    in_maps = make_in_maps(a, b)
    r = run_bass_kernel_spmd(nc, in_maps, list(range(N_CORES)))
    out = np.empty((N_CORES, ROWS_VALID), dtype=np.float32)
    for c in range(N_CORES):
        out[c] = r.results[c]["loss"][:ROWS_VALID]
    return out.reshape(B)
